# revision 13
# baseline (speedup 1.0000x reference)
"""BaseLayer MoE gate (balanced assignment) for Trainium2, 8 NeuronCores.

Strategy:
  - The roofline-dominant work is the token->expert affinity matmul
    X[16384, 2048] @ C.T[2048, 16] (reads 134 MB; the kernel is
    HBM-bandwidth bound).  Tokens are sharded 8 ways; each core computes
    aff.T[16, 2048] for its 2048-token shard.
  - Per core: X-shard is fed pre-transposed ([d_model, tok], so the
    d_model contraction lands on SBUF partitions) and streamed as
    sixteen ~1MB k-chunk DMA loads into fully-resident SBUF tiles (no
    buffer reuse -> every load is issued as early as possible and the
    stream runs gap-free at the per-core HBM cap, ~370 GB/s measured in
    a good HBM phase; run-to-run the box drifts to ~340).  The fp32
    matmul uses PE *column tiling* (tile_position=(0, 32b)) to run the
    four 512-token blocks concurrently in the four 32-column PE
    quadrants (fp32 moving costs 4 cycles/row, so without packing the
    PE would be the bottleneck).  Each block accumulates its 16
    k-chunks into its own PSUM bank, so the tail's PSUM readers are
    independent.
  - Tail: chunk 15 is split by tokens so blocks 0/1 finish and start
    evacuating before the stream ends; the four quadrant copies run on
    vector and scalar in parallel with stores issued immediately on
    both HWDGE queues (no serialized ping-pong).  Dummy matmuls after
    chunks 12-14 keep the PE at full clock through the stream tail
    (per-chunk bursts alone never satisfy the HAM activity window, so
    the final — critical-path — matmuls would run at half clock); the
    DCE-keepalive for them consumes only an early throwaway dummy so
    nothing extra lands in the tail.
  - fp32 precision end-to-end is required: the auction's final
    assignment is stable under affinity perturbations up to ~1e-6 but
    flips thousands of indices by 1e-5, which rules out bf16/fp32r
    tricks (verified empirically).
  - The auction-based balanced assignment operates on the tiny
    [16, 16384] affinity matrix and is an inherently sequential,
    data-dependent while loop (converges in ~11 iterations here); it
    runs on host as an exact bit-level replica of the reference
    semantics (verified to reproduce jax.lax.top_k tie-breaking and the
    full reference trajectory).
"""

import numpy as np

D = 2048
E = 16
N_CORES = 8
TOK_PER_CORE = 2048
N_TOK = N_CORES * TOK_PER_CORE
TOK_BLK = 512
N_BLK = TOK_PER_CORE // TOK_BLK  # 4
K_CHUNKS = D // 128  # 16

_cache = {}


def _build_nc():
    import concourse.tile as tile
    from concourse import bacc, mybir

    f32 = mybir.dt.float32

    nc = bacc.Bacc(
        "TRN2", target_bir_lowering=False, debug=False, num_devices=N_CORES
    )
    xt = nc.declare_dram_parameter("xt", [D, TOK_PER_CORE], f32, isOutput=False)
    # ctp: centroids pre-arranged on host as [128, K_CHUNKS, E]
    ctp = nc.declare_dram_parameter("ctp", [128, K_CHUNKS, E], f32, isOutput=False)
    afft = nc.declare_dram_parameter("afft", [E, TOK_PER_CORE], f32, isOutput=True)
    # internal sink that keeps the tail warm-up dummy matmuls live past DCE
    sink = nc.dram_tensor("sink", [E, TOK_BLK], f32)

    with tile.TileContext(nc) as tc:
        with tc.tile_pool(name="cpool", bufs=1) as cpool, \
             tc.tile_pool(name="hpool", bufs=1) as hpool, \
             tc.tile_pool(name="xpool", bufs=K_CHUNKS - 2) as xpool, \
             tc.tile_pool(name="lpool", bufs=2) as lpool, \
             tc.tile_pool(name="opool", bufs=4) as opool, \
             tc.tile_pool(name="psum", bufs=4, space="PSUM") as psum_pool:
            ct_sb = cpool.tile([128, K_CHUNKS, E], f32)
            nc.scalar.dma_start(out=ct_sb[:], in_=ctp[:])
            # All 16 k-chunks stay resident in SBUF (128 KB/partition),
            # so every load is issued up-front with no reuse dependency.
            # Chunk 0 gets a small 256KB head-piece so the PE warms up as
            # early as possible; chunk 15 is split by tokens so blocks
            # 0/1 finish their contraction (and start evacuating) ~1.4us
            # before the stream ends.
            xk0a = hpool.tile([128, TOK_BLK], f32, tag="xs", name="xk0a")
            nc.sync.dma_start(out=xk0a[:], in_=xt[0:128, 0:TOK_BLK])
            xk0b = hpool.tile(
                [128, TOK_PER_CORE - TOK_BLK], f32, tag="xs2", name="xk0b"
            )
            nc.sync.dma_start(out=xk0b[:], in_=xt[0:128, TOK_BLK:])
            xks = [None]
            for k in range(1, K_CHUNKS - 1):
                xk = xpool.tile([128, TOK_PER_CORE], f32, tag="xk", name=f"xk_{k}")
                nc.sync.dma_start(out=xk[:], in_=xt[k * 128:(k + 1) * 128, :])
                xks.append(xk)
            kL = K_CHUNKS - 1
            half = TOK_PER_CORE // 2
            xkLa = lpool.tile([128, half], f32, tag="xh", name="xkLa")
            nc.sync.dma_start(out=xkLa[:], in_=xt[kL * 128:, 0:half])
            # last half-chunk arrives as three pieces (512/256/256 tokens)
            # so the final blocks' matmuls start as early as possible and
            # only a 256-token piece gates the very last PE work
            xkLb = lpool.tile([128, TOK_BLK], f32, tag="xq", name="xkLb")
            nc.sync.dma_start(
                out=xkLb[:], in_=xt[kL * 128:, half:half + TOK_BLK]
            )
            xkLc = lpool.tile([128, TOK_BLK // 2], f32, tag="xo", name="xkLc")
            nc.sync.dma_start(
                out=xkLc[:],
                in_=xt[kL * 128:, half + TOK_BLK:half + TOK_BLK + TOK_BLK // 2],
            )
            xkLd = lpool.tile([128, TOK_BLK // 2], f32, tag="xo", name="xkLd")
            nc.sync.dma_start(
                out=xkLd[:], in_=xt[kL * 128:, half + TOK_BLK + TOK_BLK // 2:]
            )

            # Four separate PSUM bank tiles (one per 512-token block) so
            # the tail's PSUM readers are independent and the evacuation
            # copies run in parallel instead of being chained on one
            # tile.  Col tile b owns partitions 32b..32b+E of its bank.
            pss = [
                psum_pool.tile([128, TOK_BLK], f32, tag="ps", name=f"ps_{b}")
                for b in range(N_BLK)
            ]

            def mm(k, b, rhs):
                nc.tensor.matmul(
                    pss[b][32 * b:32 * b + E, :],
                    ct_sb[:, k, :],
                    rhs,
                    start=(k == 0), stop=(k == K_CHUNKS - 1),
                    tile_position=(0, 32 * b),
                )

            # Tail warm-up dummy matmuls: the HAM clock governor only
            # unthrottles the PE after ~3.4us of sustained activity, and
            # the per-chunk bursts here (~2.1us cold) never qualify, so
            # the final chunk's matmuls — the only ones on the critical
            # path — would run at half clock.  Dummies after chunks
            # 12-14 keep the PE busy through the stream tail so chunk 15
            # executes warm.  The DCE-keepalive copy+sink consumes only
            # the FIRST dummy tile so it retires mid-stream instead of
            # delaying the tail evacuation (scalar copy: vector is the
            # tail-critical copy engine).
            dummies = []

            mm(0, 0, xk0a[:])
            for b in range(1, N_BLK):
                mm(0, b, xk0b[:, (b - 1) * TOK_BLK:b * TOK_BLK])
            for k in range(1, K_CHUNKS - 1):
                for b in range(N_BLK):
                    mm(k, b, xks[k][:, b * TOK_BLK:(b + 1) * TOK_BLK])
                if k in (2, K_CHUNKS - 4):
                    # one warm-up burst after chunk 12 is enough: chunk
                    # 12's cold matmuls + 2 dummies give >4us sustained
                    # PE activity (HAM unthrottles), and the idle gaps
                    # between later chunks stay under the ~3.4us
                    # re-throttle window.  Dummies after 13/14 would only
                    # risk delaying chunk 15's critical matmuls.
                    ps_d = psum_pool.tile(
                        [128, TOK_BLK], f32, tag="psd", name=f"ps_d_{k}"
                    )
                    n_dum = 1 if k == 2 else 2
                    for _ in range(n_dum):
                        nc.tensor.matmul(
                            ps_d[0:E, :],
                            ct_sb[:, k, :],
                            xks[k][:, 0:TOK_BLK],
                            start=True, stop=True,
                            tile_position=(0, 0),
                        )
                    dummies.append(ps_d)
                    if len(dummies) == 1:
                        # anchor the DCE-keepalive to this early throwaway
                        # dummy (~15us in) — the unconsumed later dummies
                        # survive DCE, so nothing lands in the tail
                        sb_d = opool.tile(
                            [E, TOK_BLK], f32, tag="sbd", name="sb_d"
                        )
                        nc.scalar.copy(sb_d[:], ps_d[0:E, :])
                        nc.scalar.dma_start(out=sink[:], in_=sb_d[:])

            obs = [
                opool.tile([E, TOK_BLK], f32, tag="ob", name=f"ob_{b}")
                for b in range(N_BLK)
            ]

            def evac(b):
                # vector copies even blocks, scalar odd; stores go out on
                # sync (vector-copied) / scalar (program order after its
                # own copy) immediately.
                if b % 2 == 0:
                    nc.vector.tensor_copy(obs[b][:], pss[b][32 * b:32 * b + E, :])
                    nc.sync.dma_start(
                        out=afft[:, b * TOK_BLK:(b + 1) * TOK_BLK], in_=obs[b][:]
                    )
                else:
                    nc.scalar.copy(obs[b][:], pss[b][32 * b:32 * b + E, :])
                    nc.scalar.dma_start(
                        out=afft[:, b * TOK_BLK:(b + 1) * TOK_BLK], in_=obs[b][:]
                    )

            # blocks 0/1 finish on the first half-chunk and evacuate
            # while blocks 2/3 still stream; block 3's final accumulation
            # is two half-width matmuls so only the last 128 KB piece
            # gates the last PE work (stop=True per column range)
            mm(kL, 0, xkLa[:, 0:TOK_BLK])
            mm(kL, 1, xkLa[:, TOK_BLK:])
            evac(0)
            evac(1)
            mm(kL, 2, xkLb[:])
            evac(2)
            nc.tensor.matmul(
                pss[3][96:96 + E, 0:TOK_BLK // 2],
                ct_sb[:, kL, :], xkLc[:],
                start=False, stop=True, tile_position=(0, 96),
            )
            nc.tensor.matmul(
                pss[3][96:96 + E, TOK_BLK // 2:],
                ct_sb[:, kL, :], xkLd[:],
                start=False, stop=True, tile_position=(0, 96),
            )
            evac(3)
    nc.compile()
    return nc


def _get_nc():
    if "nc" not in _cache:
        _cache["nc"] = _build_nc()
    return _cache["nc"]


def _make_in_maps(x_flat, centroids):
    # [E, D] -> C.T [D, E] -> [K_CHUNKS, 128, E] -> [128, K_CHUNKS, E]
    ctp = np.ascontiguousarray(
        centroids.T.astype(np.float32, copy=False)
        .reshape(K_CHUNKS, 128, E)
        .transpose(1, 0, 2)
    )
    in_maps = []
    for i in range(N_CORES):
        shard = x_flat[i * TOK_PER_CORE:(i + 1) * TOK_PER_CORE]
        in_maps.append(
            {"xt": np.ascontiguousarray(shard.T), "ctp": ctp}
        )
    return in_maps


def _axon_available():
    """True if this process's jax can see the 8 NeuronCores."""
    try:
        import jax

        return len(jax.devices()) >= N_CORES and jax.default_backend() != "cpu"
    except Exception:
        return False


def _device_affinities_T(x_flat, centroids):
    """Run the 8-core bass kernel; return aff.T [E, N_TOK] float32."""
    if not _axon_available():
        return _device_affinities_T_subprocess(x_flat, centroids)
    from concourse.bass_utils import run_bass_kernel_spmd

    in_maps = _make_in_maps(x_flat, centroids)
    nc = _get_nc()
    res = run_bass_kernel_spmd(nc, in_maps, list(range(N_CORES)))
    return np.concatenate(
        [res.results[i]["afft"] for i in range(N_CORES)], axis=1
    )  # [E, N_TOK]


def _device_affinities_T_subprocess(x_flat, centroids):
    """Fallback when the calling process pinned jax to CPU: run the device
    kernel in a child process where the neuron/axon PJRT plugin can boot."""
    import os
    import subprocess
    import sys
    import tempfile

    here = os.path.dirname(os.path.abspath(__file__))
    with tempfile.TemporaryDirectory() as td:
        np.save(os.path.join(td, "x.npy"), x_flat)
        np.save(os.path.join(td, "c.npy"), centroids)
        prog = (
            "import sys, numpy as np\n"
            f"sys.path.insert(0, {here!r})\n"
            "import kernel as _k\n"
            f"x = np.load({os.path.join(td, 'x.npy')!r})\n"
            f"c = np.load({os.path.join(td, 'c.npy')!r})\n"
            "a = _k._device_affinities_T(x, c)\n"
            f"np.save({os.path.join(td, 'a.npy')!r}, a)\n"
        )
        env = dict(os.environ)
        env.pop("JAX_PLATFORMS", None)
        env["JAX_PLATFORMS"] = "axon"
        subprocess.run(
            [sys.executable, "-c", prog], env=env, check=True,
            stdout=subprocess.DEVNULL, stderr=subprocess.DEVNULL,
        )
        return np.load(os.path.join(td, "a.npy"))


def _balanced_assignment_host(s):
    """Exact host replica of the reference auction on s = scores.T [E, N]."""
    ok = np.isfinite(s)
    if not ok.all():
        fmin = np.min(np.where(ok, s, np.inf))
        s = np.where(ok, s, fmin).astype(np.float32)
    eps = np.maximum(
        np.float32((np.float32(s.max()) - np.float32(s.min())) / np.float32(50.0)),
        np.float32(1e-4),
    )
    E_, N = s.shape
    jpw = N // E_
    rows = np.arange(E_)[:, None]
    jobs_idx = np.arange(N)
    MAX_GREEDY = 100
    HARD_CAP = 200

    value = s.copy()
    cost = np.zeros(N, np.float32)
    prev_bidders = np.zeros(N, np.int32)
    prev_have = np.zeros(N, bool)
    it = 0
    top_index = None
    while it < HARD_CAP:
        order = np.argsort(-value, axis=1, kind="stable")
        top_index = order[:, : jpw + 1]
        top_values = np.take_along_axis(value, top_index, axis=1)
        bid_incr = top_values[:, :jpw] - top_values[:, jpw:] + eps
        bids = np.zeros_like(s)
        bids[rows, top_index[:, :jpw]] = bid_incr
        bids[prev_bidders, jobs_idx] = np.where(
            prev_have, eps, bids[prev_bidders, jobs_idx]
        )
        high_bids = bids.max(axis=0)
        high_bidders = bids.argmax(axis=0).astype(np.int32)
        have_bids = high_bids > 0
        done = bool(np.all(have_bids))
        cost = (cost + high_bids).astype(np.float32)
        value = (s - cost).astype(np.float32)
        if it < MAX_GREEDY:
            upd = np.full(N, np.inf, np.float32)
        else:
            upd = s[high_bidders, jobs_idx]
        value[high_bidders, jobs_idx] = np.where(
            have_bids, upd, value[high_bidders, jobs_idx]
        )
        prev_bidders = high_bidders
        prev_have = have_bids
        it += 1
        if done:
            break
    return top_index[:, :jpw].astype(np.int32)


def kernel(input_features, expert_centroids):
    x_flat = np.ascontiguousarray(
        input_features.reshape(-1, input_features.shape[-1])
    ).astype(np.float32, copy=False)
    afft = _device_affinities_T(x_flat, expert_centroids)  # [E, N]
    top_idx = _balanced_assignment_host(afft)
    top_value = np.take_along_axis(afft, top_idx, axis=1).astype(np.float32)
    return top_idx, top_value
